# revision 1
# baseline (speedup 1.0000x reference)
"""Fused sparse-attention kernel for TRN2, SPMD over 8 NeuronCores.

Sharding: data-parallel over batch (32 -> 4 per core). Per core, the full
block (LayerNorm -> fused qkv -> per-head attention with gathered relative
position bias -> proj) is computed on-chip; attention probabilities never
touch HBM.

Softmax: the reference's softmax(S + B) is computed as exp(S - c) * E with
E = exp(B) precomputed on host (B depends only on the tiny attn_biases table
and the fixed index map), and the row-sums are folded into the PV matmul via
a ones-column appended to V. No max-subtraction is needed: S is bounded
(|S| < ~12 for this distribution) and the constant offset c gives fp16
headroom; the offset cancels in the normalization.

Layouts: scores are built transposed (S.T[m, n], m on partitions) so the
exp'd probabilities can feed the PV matmul as the stationary operand without
a transpose, and the ones-column lands the softmax denominators as
per-partition scalars.
"""

import os
import sys

import numpy as np

for _p in ("/opt/trn_rl_repo", "/root/.axon_site/_ro/trn_rl_repo"):
    if os.path.isdir(_p) and _p not in sys.path:
        sys.path.insert(0, _p)

import concourse.bacc as bacc
import concourse.tile as tile
from concourse import bass_utils, mybir
from concourse.masks import make_identity

F32 = mybir.dt.float32
F16 = mybir.dt.float16

NCORES = 8
B_TOTAL = 32
NB = B_TOTAL // NCORES  # local batch per core
N = 1024
NT = 8        # 128-row tiles over n
DIM = 256
CC = 2        # 128-row chunks over DIM
H = 8
KD = 16
D = 64
MC = 8        # 128-row chunks over m
EPS = 1e-5
OFF = float(4.0 * np.log(2.0))  # exp offset for fp16 headroom (cancels)

PS_BUFS = 12
E_BUFS = 16
# (hp, mc) pairs whose E-multiply runs on GPSIMD instead of DVE
GP_TT_SET = set()  # all E-multiplies on DVE; GPSIMD only does broadcasts
DBG_G_RANGE = range(4)
DBG_B_RANGE = range(NB)
DBG_PROJ = True


DEBUG_TILES = {}


def _emit(tc, aps):
    nc = tc.nc
    x, wqk, wv, wp, bqk, bv, bp, etab, out = aps
    DEBUG_TILES.clear()

    with tc.tile_pool(name="persist", bufs=1) as persist:
        # --- constants / weights resident in SBUF ---
        wqk_sb = persist.tile([128, CC, 4, 128], F16)
        nc.sync.dma_start(out=wqk_sb, in_=wqk.rearrange("cc ci jt j -> ci cc jt j"))
        wv_sb = persist.tile([128, CC, 512], F16)
        nc.sync.dma_start(out=wv_sb, in_=wv.rearrange("cc ci v -> ci cc v"))
        wp_sb = persist.tile([128, 4, 256], F16)
        nc.sync.dma_start(out=wp_sb, in_=wp.rearrange("cc ci c -> ci cc c"))
        bqk_sb = persist.tile([128, 4], F32)
        nc.sync.dma_start(out=bqk_sb, in_=bqk.rearrange("jt j -> j jt"))
        bv_sb = persist.tile([128, 512], F32)
        nc.sync.dma_start(out=bv_sb, in_=bv.partition_broadcast(128))
        bp_sb = persist.tile([128, 256], F32)
        nc.sync.dma_start(out=bp_sb, in_=bp.partition_broadcast(128))
        ident = persist.tile([128, 128], F16)
        make_identity(nc, ident)
        negoff = persist.tile([128, 1], F32)
        nc.vector.memset(negoff, -OFF)
        epsv = persist.tile([128, 1], F32)
        nc.vector.memset(epsv, EPS)

        qkT_l = []  # per-b [128, 4, 1024] f16: jt tiles (kT g0, qT g0, kT g1, qT g1)
        v_l = []    # per-b [128, NT, H, 65] f16: V rows + ones column per head
        ot_l = []   # per-b [128, 4, 1024] f16: O.T (dh on partitions, 4 chunks)

        # ---------------- phase 1: LN, xn.T, qkv projections ----------------
        with (
            tc.tile_pool(name="p1", bufs=2) as p1,
            tc.tile_pool(name="p1ps", bufs=2, space="PSUM") as p1ps,
        ):
            for b in range(NB):
                x_sb = p1.tile([128, NT, DIM], F32, tag="x", bufs=2)
                nc.sync.dma_start(
                    out=x_sb, in_=x[b].rearrange("(t p) c -> p t c", p=128)
                )
                xn_sb = p1.tile([128, NT, DIM], F16, tag="xn", bufs=2)
                for t in range(NT):
                    stats = p1.tile([128, 6], F32, tag="stats", bufs=3)
                    nc.vector.bn_stats(out=stats, in_=x_sb[:, t])
                    mv = p1.tile([128, 2], F32, tag="mv", bufs=3)
                    nc.vector.bn_aggr(out=mv, in_=stats)
                    rstd = p1.tile([128, 1], F32, tag="rstd", bufs=3)
                    nc.scalar.activation(
                        out=rstd, in_=mv[:, 1:2],
                        func=mybir.ActivationFunctionType.Sqrt,
                        bias=epsv, scale=1.0,
                    )
                    nc.vector.reciprocal(out=rstd, in_=rstd)
                    nc.vector.tensor_scalar(
                        out=xn_sb[:, t], in0=x_sb[:, t],
                        scalar1=mv[:, 0:1], scalar2=rstd,
                        op0=mybir.AluOpType.subtract, op1=mybir.AluOpType.mult,
                    )
                # xn.T via PE transpose
                xnT = p1.tile([128, CC, N], F16, tag="xnt", bufs=2)
                for cc in range(CC):
                    for t in range(NT):
                        tp = p1ps.tile([128, 128], F16, tag="tp", bufs=2)
                        nc.tensor.transpose(
                            tp, xn_sb[:, t, cc * 128:(cc + 1) * 128], ident
                        )
                        # ScalarE is idle in phase 1; use it for the copies
                        nc.scalar.copy(
                            out=xnT[:, cc, t * 128:(t + 1) * 128], in_=tp
                        )
                # q.T / k.T, packed by 32-row strips per head (zeros padding)
                qkT = persist.tile([128, 4, N], F16, tag="qkT", bufs=NB, name="qkT")
                for jt in range(4):
                    qkp = p1ps.tile([128, N], F32, tag="qkp", bufs=2)
                    for nh in range(2):
                        for cc in range(CC):
                            nc.tensor.matmul(
                                qkp[:, nh * 512:(nh + 1) * 512],
                                lhsT=wqk_sb[:, cc, jt],
                                rhs=xnT[:, cc, nh * 512:(nh + 1) * 512],
                                start=(cc == 0), stop=(cc == CC - 1),
                            )
                    nc.scalar.activation(
                        out=qkT[:, jt], in_=qkp,
                        func=mybir.ActivationFunctionType.Identity,
                        bias=bqk_sb[:, jt:jt + 1], scale=1.0,
                    )
                qkT_l.append(qkT)
                DEBUG_TILES[("qkT", b)] = qkT
                DEBUG_TILES[("xnT", b)] = xnT
                DEBUG_TILES[("xn", b)] = xn_sb
                # V (natural layout) + ones column, interleaved per head
                v_sb = persist.tile([128, NT, H, 65], F16, tag="v", bufs=NB, name="v_sb")
                nc.vector.memset(v_sb[:, :, :, 64:65], 1.0)
                for t in range(NT):
                    vp = p1ps.tile([128, 512], F32, tag="vp", bufs=2)
                    for cc in range(CC):
                        nc.tensor.matmul(
                            vp,
                            lhsT=xnT[:, cc, t * 128:(t + 1) * 128],
                            rhs=wv_sb[:, cc],
                            start=(cc == 0), stop=(cc == CC - 1),
                        )
                    nc.vector.tensor_tensor(
                        out=v_sb[:, t, :, 0:64],
                        in0=vp.rearrange("p (h d) -> p h d", d=64),
                        in1=bv_sb.rearrange("p (h d) -> p h d", d=64),
                        op=mybir.AluOpType.add,
                    )
                v_l.append(v_sb)
                DEBUG_TILES[("v", b)] = v_sb

        # ---------------- phase 2: attention per head pair ----------------
        for b in DBG_B_RANGE:
            ot = persist.tile([128, 4, N], F16, tag="ot", bufs=NB, name="ot")
            while len(ot_l) <= b:
                ot_l.append(None)
            ot_l[b] = ot
            DEBUG_TILES[("ot", b)] = ot

        with (
            tc.tile_pool(name="p2", bufs=2) as p2,
            tc.tile_pool(name="p2ps", bufs=2, space="PSUM") as p2ps,
        ):
            for g in DBG_G_RANGE:  # head pair {2g, 2g+1}
                e_tiles = {}
                for hp in range(2):
                    for mc in range(MC):
                        et = p2.tile([128, N], F16, tag="e", bufs=E_BUFS, name="et")
                        nc.sync.dma_start(out=et, in_=etab[2 * g + hp, mc])
                        e_tiles[(hp, mc)] = et
                deferred_norm = []
                for b in DBG_B_RANGE:
                    # O'.T accumulators, one per head of the pair:
                    # [65, n] = V'.T @ P.T; row 64 carries the softmax sums
                    o_ts = [
                        p2ps.tile([65, N], F32, tag="ot", bufs=2, name="o_ts")
                        for _ in range(2)
                    ]
                    for mc in range(MC):
                        s_tiles = [
                            p2ps.tile([128, 1024], F32, tag="s", bufs=2,
                                      name="s_ps")
                            for _ in range(2)
                        ]
                        # S matmuls with strip alternation so the two heads'
                        # row-tiles execute concurrently in the PE array
                        for nh in range(2):
                            for hp in range(2):
                                h = 2 * g + hp
                                jt = 2 * (h // 4)
                                strip = 32 * (h % 4)
                                nc.tensor.matmul(
                                    s_tiles[hp][:, nh * 512:(nh + 1) * 512],
                                    lhsT=qkT_l[b][strip:strip + KD, jt,
                                                  mc * 128:(mc + 1) * 128],
                                    rhs=qkT_l[b][strip:strip + KD, jt + 1,
                                                 nh * 512:(nh + 1) * 512],
                                    start=True, stop=True,
                                    tile_position=(strip, 0),
                                )
                        ps_hp = []
                        for hp in range(2):
                            ps = p2.tile([128, 1024], F16, tag="ps", bufs=PS_BUFS,
                                         name="ps")
                            nc.scalar.activation(
                                out=ps, in_=s_tiles[hp],
                                func=mybir.ActivationFunctionType.Exp,
                                bias=negoff, scale=1.0,
                            )
                            eng = nc.gpsimd if (hp, mc) in GP_TT_SET else nc.vector
                            eng.tensor_tensor(
                                out=ps, in0=ps, in1=e_tiles[(hp, mc)],
                                op=mybir.AluOpType.mult,
                            )
                            ps_hp.append(ps)
                        # PV: V' (with ones column) stationary, P.T streaming
                        for hp in range(2):
                            for nh in range(2):
                                nc.tensor.matmul(
                                    o_ts[hp][:, nh * 512:(nh + 1) * 512],
                                    lhsT=v_l[b][:, mc, 2 * g + hp],
                                    rhs=ps_hp[hp][:, nh * 512:(nh + 1) * 512],
                                    start=(mc == 0), stop=(mc == MC - 1),
                                    skip_group_check=True,
                                )
                    # Drain PSUM immediately (single ScalarE copy) so the o_ts
                    # slots free fast; recip + broadcast are issued now but the
                    # normalize multiplies are DEFERRED to the end of the g
                    # group so slow broadcasts never block the DVE queue.
                    for hp in range(2):
                        raw = p2.tile([65, N], F16, tag="raw", bufs=9)
                        nc.scalar.copy(out=raw, in_=o_ts[hp])
                        r1 = p2.tile([1, N], F16, tag="r1", bufs=4)
                        with nc.allow_low_precision(
                            reason="1/sums is in normal fp16 range (exp offset)"
                        ):
                            nc.vector.reciprocal(out=r1, in_=raw[64:65, :])
                        rb = p2.tile([64, N], F16, tag="rb", bufs=9)
                        nc.gpsimd.partition_broadcast(rb, r1)
                        deferred_norm.append((b, hp, raw, rb))
                for db, dhp, raw, rb in deferred_norm:
                    nc.vector.tensor_tensor(
                        out=ot_l[db][64 * dhp:64 * dhp + 64, g, :],
                        in0=raw[0:64, :],
                        in1=rb,
                        op=mybir.AluOpType.mult,
                    )

        # ---------------- phase 3: output projection ----------------
        if not DBG_PROJ:
            return
        with (
            tc.tile_pool(name="p3", bufs=2) as p3,
            tc.tile_pool(name="p3ps", bufs=4, space="PSUM") as p3ps,
        ):
            for b in DBG_B_RANGE:
                o_sb = p3.tile([128, NT, 256], F32, tag="osb", bufs=2)
                for nt in range(NT):
                    y = p3ps.tile([128, 256], F32, tag="y", bufs=4)
                    for cc2 in range(4):
                        nc.tensor.matmul(
                            y,
                            lhsT=ot_l[b][:, cc2, nt * 128:(nt + 1) * 128],
                            rhs=wp_sb[:, cc2],
                            start=(cc2 == 0), stop=(cc2 == 3),
                        )
                    nc.vector.tensor_tensor(
                        out=o_sb[:, nt], in0=y, in1=bp_sb, op=mybir.AluOpType.add
                    )
                nc.sync.dma_start(
                    out=out[b].rearrange("(t p) c -> p t c", p=128), in_=o_sb
                )


def build_module():
    nc = bacc.Bacc(
        "TRN2",
        target_bir_lowering=False,
        debug=False,
        enable_asserts=False,
        num_devices=NCORES,
    )
    x_t = nc.dram_tensor("x", [NB, N, DIM], F32, kind="ExternalInput")
    wqk_t = nc.dram_tensor("wqk", [CC, 128, 4, 128], F16, kind="ExternalInput")
    wv_t = nc.dram_tensor("wv", [CC, 128, 512], F16, kind="ExternalInput")
    wp_t = nc.dram_tensor("wp", [4, 128, 256], F16, kind="ExternalInput")
    bqk_t = nc.dram_tensor("bqk", [4, 128], F32, kind="ExternalInput")
    bv_t = nc.dram_tensor("bv", [512], F32, kind="ExternalInput")
    bp_t = nc.dram_tensor("bp", [256], F32, kind="ExternalInput")
    e_t = nc.dram_tensor("etab", [H, MC, 128, N], F16, kind="ExternalInput")
    out_t = nc.dram_tensor("out", [NB, N, DIM], F32, kind="ExternalOutput")

    aps = [t.ap() for t in (x_t, wqk_t, wv_t, wp_t, bqk_t, bv_t, bp_t, e_t, out_t)]
    with tile.TileContext(nc) as tc:
        _emit(tc, aps)
    nc.compile()
    return nc


def prep_inputs(inputs):
    """Host-side prep: fold norm affine + scale into weights, pack q/k rows
    into 32-row strips for PE row-tiling, and materialize E = exp(bias)."""
    x = np.asarray(inputs["x"], np.float32)
    norm_w = np.asarray(inputs["norm_w"], np.float32)
    norm_b = np.asarray(inputs["norm_b"], np.float32)
    qkv_w = np.asarray(inputs["qkv_w"], np.float32)
    qkv_b = np.asarray(inputs["qkv_b"], np.float32)
    proj_w = np.asarray(inputs["proj_w"], np.float32)
    proj_b = np.asarray(inputs["proj_b"], np.float32)
    ab = np.asarray(inputs["attn_biases"], np.float32)
    bi = np.asarray(inputs["bias_idxs"], np.int64)

    scale = KD ** -0.5
    wr = qkv_w.reshape(H, 2 * KD + D, DIM)
    br = qkv_b.reshape(H, 2 * KD + D)
    # fold norm_w into weights, norm_b into biases
    w_eff = wr * norm_w[None, None, :]
    b_eff = br + wr @ norm_b
    w_q = w_eff[:, :KD] * scale
    b_q = b_eff[:, :KD] * scale
    w_k = w_eff[:, KD:2 * KD]
    b_k = b_eff[:, KD:2 * KD]
    w_v = w_eff[:, 2 * KD:]
    b_v = b_eff[:, 2 * KD:]

    wqk = np.zeros((CC, 128, 4, 128), np.float16)
    bqk = np.zeros((4, 128), np.float32)
    for jt in range(4):
        kind_q = jt % 2 == 1
        hg = jt // 2
        w_src = w_q if kind_q else w_k
        b_src = b_q if kind_q else b_k
        for hp in range(4):
            h = hg * 4 + hp
            w_jc = w_src[h]  # [KD, DIM]
            for cc in range(CC):
                wqk[cc, :, jt, 32 * hp:32 * hp + KD] = (
                    w_jc[:, cc * 128:(cc + 1) * 128].T.astype(np.float16)
                )
            bqk[jt, 32 * hp:32 * hp + KD] = b_src[h]

    wv = np.zeros((CC, 128, 512), np.float16)
    for cc in range(CC):
        # [512(h,d), 128] -> [128, 512]
        wv[cc] = w_v.reshape(512, DIM)[:, cc * 128:(cc + 1) * 128].T.astype(np.float16)
    bv = b_v.reshape(512).astype(np.float32)

    wp = np.zeros((4, 128, 256), np.float16)
    for cc2 in range(4):
        wp[cc2] = proj_w[:, cc2 * 128:(cc2 + 1) * 128].T.astype(np.float16)
    bp = proj_b.astype(np.float32)

    etab = np.exp(ab[:, bi]).astype(np.float16).reshape(H, MC, 128, N)

    shared = {
        "wqk": wqk, "wv": wv, "wp": wp,
        "bqk": bqk, "bv": bv, "bp": bp, "etab": etab,
    }
    in_maps = []
    for c in range(NCORES):
        m = dict(shared)
        m["x"] = np.ascontiguousarray(x[c * NB:(c + 1) * NB])
        in_maps.append(m)
    return in_maps


_NC_CACHE = None


def _get_nc():
    global _NC_CACHE
    if _NC_CACHE is None:
        _NC_CACHE = build_module()
    return _NC_CACHE


def run(inputs, **spmd_kwargs):
    nc = _get_nc()
    in_maps = prep_inputs(inputs)
    res = bass_utils.run_bass_kernel_spmd(
        nc, in_maps, core_ids=list(range(NCORES)), **spmd_kwargs
    )
    out = np.concatenate([res.results[c]["out"] for c in range(NCORES)], axis=0)
    return out.astype(np.float32), res


def kernel(**inputs):
    out, _ = run(inputs)
    return out


if __name__ == "__main__":
    print("building module...")
    nc = _get_nc()
    print("instructions:", sum(len(f.basicblocks[0].instructions)
                               for f in nc.m.functions if f.basicblocks))

